# revision 11
# baseline (speedup 1.0000x reference)
"""Self-contained Trainium2 Bass kernel for nn_Classifier_6682969112951.

Model (per batch b): 4-head additive attention pooling over S=300 tokens
(D=1024 -> P=128 projections), conv classifier (C=3862), layernorm over C,
max/argmax over heads, sigmoid. Outputs:
  (vid_probs [64,3862] f32, attn_idc [64,3862] i32,
   scores [64,300,4] f32, attn_weights [64,300,4] f32, conv_loss scalar f32)

Sharding: data-parallel over batch across 8 NeuronCores (8 batches/core),
weights replicated. conv_loss depends only on (Wc, bc) -> host numpy.

Device strategy (bf16 matmuls, fp32 accumulation / softmax / layernorm):
  - X^T arrives via hardware DMA-transpose from DRAM (host pre-casts to
    bf16 and pads (b,s)=2400 rows -> 2432).
  - Projections: stationary W^T chunks [128d,128p], moving X^T, accumulated
    over 8 d-chunks in PSUM, drained with fused bias add.
  - score = sum_p (q+bq)*keys: elementwise product (DVE) then ones-matmul
    partition reduction; the 4 head rows land at PSUM partitions {0,32,64,96}
    and are compacted to one [32(hb),300] tile by an SBUF->SBUF DMA.
  - softmax on [32,300]; ws via per-batch matmuls against transposed vals;
    logits directly in [c,bh] layout (Wc^T chunks stationary); LN stats via
    DVE pre-reduce + ones-matmul partition sums; normalize with stride-0
    broadcast; max/argmax over heads via is_ge arithmetic; sigmoid via
    exp + reciprocal (keeps ACT on one table set).
  - probs/idx leave the device in [c_lo, ch, b] layout; host permutes.
"""
import math

import numpy as np

B, S, D, P, H, C = 64, 300, 1024, 128, 4, 3862
NCORES = 8
BPC = B // NCORES          # batches per core
BS = BPC * S               # 2400
BSP = 2432                 # padded (b,s) extent: 19*128, multiple of 16
CP = 3968                  # padded C: 31*128
NCH = CP // 128            # 31 c-chunks
SCALE = 1.0 / math.sqrt(P)
EPS = 1e-6

_CACHE = {}


def _build_program():
    import concourse.mybir as mybir
    import concourse.tile as tile
    from concourse import bacc

    F32 = mybir.dt.float32
    BF16 = mybir.dt.bfloat16
    I32 = mybir.dt.int32
    AX = mybir.AxisListType
    ALU = mybir.AluOpType
    ACTF = mybir.ActivationFunctionType

    nc = bacc.Bacc("TRN2", target_bir_lowering=False, debug=False)

    xT_d = nc.dram_tensor("xT_in", [BSP, D], BF16, kind="ExternalInput").ap()
    wT_d = nc.dram_tensor("wT_in", [8, 128, 6, 128], BF16, kind="ExternalInput").ap()
    wcT_d = nc.dram_tensor("wcT_in", [128, CP], BF16, kind="ExternalInput").ap()
    bqT_d = nc.dram_tensor("bqT_in", [128, H], F32, kind="ExternalInput").ap()
    bk_d = nc.dram_tensor("bk_in", [128, 1], F32, kind="ExternalInput").ap()
    bv_d = nc.dram_tensor("bv_in", [128, 1], F32, kind="ExternalInput").ap()
    bcT_d = nc.dram_tensor("bcT_in", [128, NCH], F32, kind="ExternalInput").ap()
    id_d = nc.dram_tensor("id_in", [128, 128], BF16, kind="ExternalInput").ap()
    ones_d = nc.dram_tensor("ones_in", [128, 32], BF16, kind="ExternalInput").ap()
    onesf_d = nc.dram_tensor("onesf_in", [128, 1], F32, kind="ExternalInput").ap()
    onesrow_d = nc.dram_tensor("onesrow_in", [1, 128], F32, kind="ExternalInput").ap()

    sc_o = nc.dram_tensor("scores_out", [32, S], F32, kind="ExternalOutput").ap()
    aw_o = nc.dram_tensor("attnw_out", [32, S], F32, kind="ExternalOutput").ap()
    pr_o = nc.dram_tensor("probs_out", [128, NCH, BPC], F32, kind="ExternalOutput").ap()
    ix_o = nc.dram_tensor("idx_out", [128, NCH, BPC], I32, kind="ExternalOutput").ap()

    THIRDS = ((0, 1024), (1024, 1024), (2048, 384))
    SCH = ((0, 128), (128, 128), (256, 44))  # per-batch s chunks

    with tile.TileContext(nc) as tc:
        with (
            tc.tile_pool(name="const", bufs=1) as cpool,
            tc.tile_pool(name="big", bufs=1) as big,
            tc.tile_pool(name="work", bufs=2) as work,
            tc.tile_pool(name="small", bufs=2) as small,
        ):
            # ---- constant / input loads (all plain copies first; the
            # X^T DMA-transposes run uninterrupted after them to avoid
            # xbar-mode serialization) ----
            wt = cpool.tile([128, 8, 6, 128], BF16)
            nc.gpsimd.dma_start(out=wt, in_=wT_d.rearrange("a b c d -> b a c d"))
            bqT = cpool.tile([128, H], F32)
            nc.gpsimd.dma_start(out=bqT, in_=bqT_d)
            bk = cpool.tile([128, 1], F32)
            nc.gpsimd.dma_start(out=bk, in_=bk_d)
            bv = cpool.tile([128, 1], F32)
            nc.gpsimd.dma_start(out=bv, in_=bv_d)
            ident = cpool.tile([128, 128], BF16)
            nc.gpsimd.dma_start(out=ident, in_=id_d)
            ones_bf = cpool.tile([128, 32], BF16)
            nc.gpsimd.dma_start(out=ones_bf, in_=ones_d)
            ones_f = cpool.tile([128, 1], F32)
            nc.gpsimd.dma_start(out=ones_f, in_=onesf_d)
            onesrow = cpool.tile([1, 128], F32)
            nc.gpsimd.dma_start(out=onesrow, in_=onesrow_d)
            wct = cpool.tile([128, CP], BF16)
            nc.gpsimd.dma_start(out=wct, in_=wcT_d)
            bcT = cpool.tile([128, NCH], F32)
            nc.gpsimd.dma_start(out=bcT, in_=bcT_d)
            # X^T per-third tiles (separate tiles -> no false deps against
            # projection reads of earlier thirds), third-major order
            xts = []
            for ti, (base, size) in enumerate(THIRDS):
                xt_t = big.tile([128, 8, size], BF16, name=f"xt{ti}")
                xts.append(xt_t)
                for dc in range(8):
                    nc.sync.dma_start(
                        out=xt_t[:, dc, :],
                        in_=xT_d[base:base + size, dc * 128:(dc + 1) * 128],
                        transpose=True,
                    )

            keys_bf = big.tile([128, BSP], BF16)
            vals_bf = big.tile([128, BSP], BF16)
            prod_bf = big.tile([128, H, BSP], BF16)
            score_wide = big.tile([128, BSP], F32)

            with (
                tc.tile_pool(name="projps", bufs=2, space="PSUM") as projps,
                tc.tile_pool(name="scoreps", bufs=2, space="PSUM") as scoreps,
            ):
                def proj_part(j, ti, size, pp):
                    for dc in range(8):
                        for lo in range(0, size, 512):
                            hi = min(lo + 512, size)
                            nc.tensor.matmul(
                                pp[:, lo:hi],
                                wt[:, dc, j, :],
                                xts[ti][:, dc, lo:hi],
                                start=(dc == 0), stop=(dc == 7),
                            )

                # projections, third-major: keys, vals, then q heads
                for ti, (base, size) in enumerate(THIRDS):
                    for j in range(6):
                        pp = projps.tile([128, 1024], F32, tag="proj")
                        proj_part(j, ti, size, pp)
                        if j == 0:
                            nc.scalar.activation(
                                keys_bf[:, base:base + size], pp[:, 0:size],
                                ACTF.Identity, bias=bk)
                        elif j == 1:
                            nc.scalar.activation(
                                vals_bf[:, base:base + size], pp[:, 0:size],
                                ACTF.Identity, bias=bv)
                        else:
                            h = j - 2
                            nc.vector.scalar_tensor_tensor(
                                out=prod_bf[:, h, base:base + size],
                                in0=pp[:, 0:size],
                                scalar=bqT[:, h:h + 1],
                                in1=keys_bf[:, base:base + size],
                                op0=ALU.add, op1=ALU.mult,
                            )

                # vals^T for all batches (overlaps on PE)
                vn_all = big.tile([128, BPC, 3, 128], BF16)
                for b in range(BPC):
                    vn_ps = projps.tile([128, 3, 128], BF16, tag="vnps",
                                        bufs=1)
                    for k, (lo, sz) in enumerate(SCH):
                        nc.tensor.matmul(
                            vn_ps[0:sz, k, :],
                            vals_bf[:, b * S + lo: b * S + lo + sz],
                            ident,
                            is_transpose=True, start=True, stop=True,
                        )
                    nc.vector.tensor_copy(
                        vn_all[:, b, 0:2, :], vn_ps[:, 0:2, :])
                    nc.vector.tensor_copy(
                        vn_all[0:44, b, 2, :], vn_ps[0:44, 2, :])

                # score: partition-reduce products via ones-matmul.
                # PE can only place outputs at partitions {0,32,64}; head 3
                # goes to its own tile and is drained into row 96.
                for ci in range(0, BSP, 512):
                    ce = min(ci + 512, BSP)
                    sp = scoreps.tile([128, 512], F32, tag="score")
                    spb = scoreps.tile([32, 512], F32, tag="scoreb", bufs=1)
                    for h in range(3):
                        nc.tensor.matmul(
                            sp[32 * h:32 * h + 32, 0:ce - ci],
                            ones_bf,
                            prod_bf[:, h, ci:ce],
                            start=True, stop=True,
                        )
                    nc.tensor.matmul(
                        spb[:, 0:ce - ci], ones_bf, prod_bf[:, 3, ci:ce],
                        start=True, stop=True,
                    )
                    nc.vector.tensor_copy(
                        score_wide[0:65, ci:ce], sp[0:65, 0:ce - ci])
                    nc.vector.tensor_copy(
                        score_wide[96:97, ci:ce], spb[0:1, 0:ce - ci])

            # compact rows {0,32,64,96} x [b, s] -> [hb, s] (hb = h*BPC + b)
            score_hb = small.tile([32, S], F32, tag="shb")
            nc.sync.dma_start(
                out=score_hb,
                in_=score_wide[0::32, 0:BS].rearrange("h (b s) -> h b s", b=BPC),
            )

            # ---- softmax over s (free axis) ----
            lnwarm = small.tile([1, 1], F32, tag="lnwarm")
            nc.scalar.activation(lnwarm, onesrow[:, 0:1], ACTF.Ln)
            mx = small.tile([32, 1], F32, tag="mx")
            nc.vector.tensor_reduce(mx, score_hb, axis=AX.X, op=ALU.max)
            nbias = small.tile([32, 1], F32, tag="nb")
            nc.vector.tensor_scalar_mul(nbias, mx, -SCALE)
            expv = small.tile([32, S], F32, tag="expv")
            denom = small.tile([32, 1], F32, tag="den")
            nc.scalar.activation(
                expv, score_hb, ACTF.Exp, bias=nbias, scale=SCALE,
                accum_out=denom,
            )
            rden = small.tile([32, 1], F32, tag="rden")
            nc.vector.reciprocal(rden, denom)
            p_attn = small.tile([32, S], F32, tag="pattn")
            nc.vector.tensor_scalar_mul(p_attn, expv, rden)
            p_attn_bf = small.tile([32, S], BF16, tag="pattnb")
            nc.vector.tensor_copy(p_attn_bf, p_attn)
            score_sc = small.tile([32, S], F32, tag="scsc")
            nc.scalar.mul(score_sc, score_hb, SCALE)
            nc.sync.dma_start(out=sc_o, in_=score_sc)
            nc.sync.dma_start(out=aw_o, in_=p_attn)

            with (
                tc.tile_pool(name="tps", bufs=1, space="PSUM") as tps,
                tc.tile_pool(name="wps", bufs=2, space="PSUM") as wps,
            ):
                # ---- p_attn^T: [32, s] -> [s, 32] (3 chunks) ----
                pat_ps = tps.tile([128, 3, 32], BF16, tag="patps")
                for k, (lo, sz) in enumerate(SCH):
                    nc.tensor.matmul(
                        pat_ps[0:sz, k, :],
                        p_attn_bf[:, lo:lo + sz],
                        ident[0:32, 0:32],
                        is_transpose=True, start=True, stop=True,
                    )
                p_attn_T = small.tile([128, 3, 32], BF16, tag="patT")
                nc.vector.tensor_copy(p_attn_T[:, 0:2, :], pat_ps[:, 0:2, :])
                nc.vector.tensor_copy(p_attn_T[0:44, 2, :], pat_ps[0:44, 2, :])

                # ---- ws per batch (vals already transposed) ----
                ws_T = cpool.tile([128, 32], BF16)   # columns bh = 4*b + h
                for b in range(BPC):
                    wp = wps.tile([128, 32], F32, tag="wsps")
                    for k, (lo, sz) in enumerate(SCH):
                        nc.tensor.matmul(
                            wp[:, 0:4],
                            vn_all[0:sz, b, k, :],
                            p_attn_T[0:sz, k, b::BPC],
                            start=(k == 0), stop=(k == 2),
                        )
                    nc.scalar.activation(
                        ws_T[:, 4 * b:4 * b + 4], wp[:, 0:4], ACTF.Relu)

                # ---- logits directly in [c, bh] ----
                logitsT = big.tile([128, NCH, 32], F32)
                for ct in range(NCH):
                    lp = wps.tile([128, 32], F32, tag="lgps")
                    nc.tensor.matmul(
                        lp, wct[:, ct * 128:(ct + 1) * 128], ws_T,
                        start=True, stop=True,
                    )
                    nc.vector.tensor_scalar_add(
                        logitsT[:, ct, :], lp, bcT[:, ct:ct + 1])

                # ---- layernorm over c ----
                rowsum = work.tile([128, 32], F32, tag="rsum")
                nc.vector.tensor_reduce(
                    rowsum, logitsT.rearrange("p a b -> p b a"),
                    axis=AX.X, op=ALU.add)
                msum = tps.tile([1, 32], F32, tag="stat")
                nc.tensor.matmul(msum, ones_f, rowsum, start=True, stop=True)
                mean = small.tile([1, 32], F32, tag="mean")
                nc.vector.tensor_scalar_mul(mean, msum, 1.0 / C)
                mb_ps = tps.tile([128, 32], F32, tag="bcast")
                nc.tensor.matmul(mb_ps, onesrow, mean, start=True, stop=True)
                mean_b = work.tile([128, 32], F32, tag="meanb")
                nc.vector.tensor_copy(mean_b, mb_ps)
                nc.vector.tensor_tensor(
                    out=logitsT, in0=logitsT,
                    in1=mean_b.unsqueeze(1).broadcast_to([128, NCH, 32]),
                    op=ALU.subtract)
                sq = big.tile([128, NCH, 32], F32)
                nc.vector.tensor_tensor(out=sq, in0=logitsT, in1=logitsT,
                                        op=ALU.mult)
                sqsum = work.tile([128, 32], F32, tag="sqsum")
                nc.vector.tensor_reduce(
                    sqsum, sq.rearrange("p a b -> p b a"),
                    axis=AX.X, op=ALU.add)
                vsum = tps.tile([1, 32], F32, tag="stat")
                nc.tensor.matmul(vsum, ones_f, sqsum, start=True, stop=True)
                # pad rows (CP - C of them) each contributed (0 - mean)^2
                m2 = small.tile([1, 32], F32, tag="m2")
                nc.vector.tensor_mul(m2, mean, mean)
                nc.vector.tensor_scalar_mul(m2, m2, float(CP - C))
                ssc = small.tile([1, 32], F32, tag="ssc")
                nc.vector.tensor_sub(ssc, vsum, m2)
                var = small.tile([1, 32], F32, tag="var")
                nc.vector.tensor_scalar_mul(var, ssc, 1.0 / (C - 1))
                # std = exp(0.5*ln(var)); inv = 1/(std + eps)
                lnv = small.tile([1, 32], F32, tag="lnv")
                nc.scalar.activation(lnv, var, ACTF.Ln)
                std = small.tile([1, 32], F32, tag="std")
                nc.scalar.activation(std, lnv, ACTF.Exp, scale=0.5)
                stde = small.tile([1, 32], F32, tag="stde")
                nc.vector.tensor_scalar_add(stde, std, EPS)
                inv = small.tile([1, 32], F32, tag="inv")
                nc.vector.reciprocal(inv, stde)
                ib_ps = tps.tile([128, 32], F32, tag="bcast")
                nc.tensor.matmul(ib_ps, onesrow, inv, start=True, stop=True)
                inv_b = work.tile([128, 32], F32, tag="invb")
                nc.vector.tensor_copy(inv_b, ib_ps)
                nc.vector.tensor_tensor(
                    out=logitsT, in0=logitsT,
                    in1=inv_b.unsqueeze(1).broadcast_to([128, NCH, 32]),
                    op=ALU.mult)

            # ---- max / argmax over h; sigmoid ----
            zv = logitsT.rearrange("p a (b h) -> p a b h", h=H)
            m = work.tile([128, NCH, BPC], F32, tag="m")
            nc.vector.tensor_reduce(m, zv, axis=AX.X, op=ALU.max)
            ge = big.tile([128, NCH, BPC, H], F32)
            for h in range(3):
                nc.vector.tensor_tensor(
                    out=ge[:, :, :, h], in0=zv[:, :, :, h], in1=m, op=ALU.is_ge)
                nc.vector.tensor_scalar_mul(
                    ge[:, :, :, h], ge[:, :, :, h], float(3 - h))
            r = work.tile([128, NCH, BPC], F32, tag="r")
            nc.vector.tensor_reduce(
                r, ge[:, :, :, 0:3], axis=AX.X, op=ALU.max)
            idx = work.tile([128, NCH, BPC], I32, tag="idx")
            nc.scalar.activation(idx, r, ACTF.Copy, scale=-1.0, bias=3.0)
            nc.sync.dma_start(out=ix_o, in_=idx)
            # sigmoid(m) = 1 / (1 + exp(-m))
            em = work.tile([128, NCH, BPC], F32, tag="em")
            nc.scalar.activation(em, m, ACTF.Exp, scale=-1.0)
            nc.vector.tensor_scalar_add(em, em, 1.0)
            probs = work.tile([128, NCH, BPC], F32, tag="probs")
            nc.vector.reciprocal(probs, em)
            nc.sync.dma_start(out=pr_o, in_=probs)

    nc.compile()
    return nc


def _get_program():
    if "nc" not in _CACHE:
        _CACHE["nc"] = _build_program()
    return _CACHE["nc"]


def _prep_inputs(seg_features, Wq, bq, Wk, bk, Wv, bv, Wc, bc):
    import ml_dtypes

    BF = ml_dtypes.bfloat16
    # stacked weights, order [keys, vals, q0..q3]; W^T layout [8, 128, 6, 128]
    wstack = np.stack([Wk, Wv, Wq[0], Wq[1], Wq[2], Wq[3]], axis=0)
    wT = np.ascontiguousarray(wstack.transpose(2, 0, 1))   # [1024, 6, 128]
    wT = wT.reshape(8, 128, 6, 128).astype(BF)
    wcT = np.zeros((128, CP), np.float32)
    wcT[:, :C] = Wc.T
    wcT = wcT.astype(BF)
    bcT = np.zeros((CP,), np.float32)
    bcT[:C] = bc
    bcT = np.ascontiguousarray(bcT.reshape(NCH, 128).T)
    common = {
        "wT_in": wT,
        "wcT_in": wcT,
        "bqT_in": np.ascontiguousarray(bq.T).astype(np.float32),
        "bk_in": bk.reshape(128, 1).astype(np.float32),
        "bv_in": bv.reshape(128, 1).astype(np.float32),
        "bcT_in": bcT.astype(np.float32),
        "id_in": np.eye(128, dtype=np.float32).astype(BF),
        "ones_in": np.ones((128, 32), np.float32).astype(BF),
        "onesf_in": np.ones((128, 1), np.float32),
        "onesrow_in": np.ones((1, 128), np.float32),
    }
    in_maps = []
    for i in range(NCORES):
        xs = seg_features[i * BPC:(i + 1) * BPC].reshape(BS, D)
        xp = np.zeros((BSP, D), np.float32)
        xp[:BS] = xs
        m = dict(common)
        m["xT_in"] = xp.astype(BF)
        in_maps.append(m)
    return in_maps


def _gather(res):
    vid_probs = np.empty((B, C), np.float32)
    attn_idc = np.empty((B, C), np.int32)
    scores = np.empty((B, S, H), np.float32)
    attn_w = np.empty((B, S, H), np.float32)
    for i, r in enumerate(res):
        sl = slice(i * BPC, (i + 1) * BPC)
        vid_probs[sl] = r["probs_out"].transpose(2, 1, 0).reshape(BPC, CP)[:, :C]
        attn_idc[sl] = r["idx_out"].transpose(2, 1, 0).reshape(BPC, CP)[:, :C]
        scores[sl] = r["scores_out"].reshape(H, BPC, S).transpose(1, 2, 0)
        attn_w[sl] = r["attnw_out"].reshape(H, BPC, S).transpose(1, 2, 0)
    return vid_probs, attn_idc, scores, attn_w


def _conv_loss(Wc, bc):
    cp = (Wc.sum(axis=-1) + bc).astype(np.float64)
    cp = cp - cp.max()
    e = np.exp(cp)
    p = e / e.sum()
    stdv = math.sqrt(float(((p - p.mean()) ** 2).sum()) / (C - 1))
    return np.float32(B * min(max(stdv, 1e-9), 1e9))


def kernel(seg_features, Wq, bq, Wk, bk, Wv, bv, Wc, bc, ln_a, ln_b,
           _trace=False):
    from concourse import bass_utils

    seg_features = np.asarray(seg_features, np.float32)
    Wq = np.asarray(Wq, np.float32)
    bq = np.asarray(bq, np.float32)
    Wk = np.asarray(Wk, np.float32)
    bk = np.asarray(bk, np.float32)
    Wv = np.asarray(Wv, np.float32)
    bv = np.asarray(bv, np.float32)
    Wc = np.asarray(Wc, np.float32)
    bc = np.asarray(bc, np.float32)
    assert np.all(np.asarray(ln_a) == 1.0) and np.all(np.asarray(ln_b) == 0.0), (
        "device fast-path assumes identity layernorm affine"
    )

    nc = _get_program()
    in_maps = _prep_inputs(seg_features, Wq, bq, Wk, bk, Wv, bv, Wc, bc)
    res = bass_utils.run_bass_kernel_spmd(
        nc, in_maps, core_ids=list(range(NCORES)), trace=_trace)

    vid_probs, attn_idc, scores, attn_w = _gather(res.results)
    out = (vid_probs, attn_idc, scores, attn_w, _conv_loss(Wc, bc))
    if _trace:
        return out, res
    return out


# revision 12
# speedup vs baseline: 1.0784x; 1.0784x over previous
"""Self-contained Trainium2 Bass kernel for nn_Classifier_6682969112951.

Model (per batch b): 4-head additive attention pooling over S=300 tokens
(D=1024 -> P=128 projections), conv classifier (C=3862), layernorm over C,
max/argmax over heads, sigmoid. Outputs:
  (vid_probs [64,3862] f32, attn_idc [64,3862] i32,
   scores [64,300,4] f32, attn_weights [64,300,4] f32, conv_loss scalar f32)

Sharding: data-parallel over batch across 8 NeuronCores (8 batches/core),
weights replicated. conv_loss depends only on (Wc, bc) -> host numpy.

Device strategy (bf16 matmuls, fp32 accumulation / softmax / layernorm):
  - X^T arrives via hardware DMA-transpose from DRAM (host pre-casts to
    bf16 and pads (b,s)=2400 rows -> 2432).
  - Projections: stationary W^T chunks [128d,128p], moving X^T, accumulated
    over 8 d-chunks in PSUM, drained with fused bias add.
  - score = sum_p (q+bq)*keys: elementwise product (DVE) then ones-matmul
    partition reduction; the 4 head rows land at PSUM partitions {0,32,64,96}
    and are compacted to one [32(hb),300] tile by an SBUF->SBUF DMA.
  - softmax on [32,300]; ws via per-batch matmuls against transposed vals;
    logits directly in [c,bh] layout (Wc^T chunks stationary); LN stats via
    DVE pre-reduce + ones-matmul partition sums; normalize with stride-0
    broadcast; max/argmax over heads via is_ge arithmetic; sigmoid via
    exp + reciprocal (keeps ACT on one table set).
  - probs/idx leave the device in [c_lo, ch, b] layout; host permutes.
"""
import math

import numpy as np

B, S, D, P, H, C = 64, 300, 1024, 128, 4, 3862
NCORES = 8
BPC = B // NCORES          # batches per core
BS = BPC * S               # 2400
BSP = 2432                 # padded (b,s) extent: 19*128, multiple of 16
CP = 3968                  # padded C: 31*128
NCH = CP // 128            # 31 c-chunks
SCALE = 1.0 / math.sqrt(P)
EPS = 1e-6

_CACHE = {}


def _build_program():
    import concourse.mybir as mybir
    import concourse.tile as tile
    from concourse import bacc

    F32 = mybir.dt.float32
    BF16 = mybir.dt.bfloat16
    I32 = mybir.dt.int32
    AX = mybir.AxisListType
    ALU = mybir.AluOpType
    ACTF = mybir.ActivationFunctionType

    nc = bacc.Bacc("TRN2", target_bir_lowering=False, debug=False)

    xT_d = nc.dram_tensor("xT_in", [BSP, D], BF16, kind="ExternalInput").ap()
    wT_d = nc.dram_tensor("wT_in", [8, 128, 6, 128], BF16, kind="ExternalInput").ap()
    wcT_d = nc.dram_tensor("wcT_in", [128, CP], BF16, kind="ExternalInput").ap()
    bqT_d = nc.dram_tensor("bqT_in", [128, H], F32, kind="ExternalInput").ap()
    bk_d = nc.dram_tensor("bk_in", [128, 1], F32, kind="ExternalInput").ap()
    bv_d = nc.dram_tensor("bv_in", [128, 1], F32, kind="ExternalInput").ap()
    bcT_d = nc.dram_tensor("bcT_in", [128, NCH], F32, kind="ExternalInput").ap()
    id_d = nc.dram_tensor("id_in", [128, 128], BF16, kind="ExternalInput").ap()
    ones_d = nc.dram_tensor("ones_in", [128, 32], BF16, kind="ExternalInput").ap()
    onesf_d = nc.dram_tensor("onesf_in", [128, 1], F32, kind="ExternalInput").ap()
    onesrow_d = nc.dram_tensor("onesrow_in", [1, 128], F32, kind="ExternalInput").ap()

    sc_o = nc.dram_tensor("scores_out", [32, S], F32, kind="ExternalOutput").ap()
    aw_o = nc.dram_tensor("attnw_out", [32, S], F32, kind="ExternalOutput").ap()
    pr_o = nc.dram_tensor("probs_out", [128, NCH, BPC], F32, kind="ExternalOutput").ap()
    ix_o = nc.dram_tensor("idx_out", [128, NCH, BPC], I32, kind="ExternalOutput").ap()

    HALVES = ((0, 1216), (1216, 1216))
    SCH = ((0, 128), (128, 128), (256, 44))  # per-batch s chunks

    with tile.TileContext(nc) as tc:
        with (
            tc.tile_pool(name="const", bufs=1) as cpool,
            tc.tile_pool(name="big", bufs=1) as big,
            tc.tile_pool(name="work", bufs=2) as work,
            tc.tile_pool(name="small", bufs=2) as small,
        ):
            # ---- constant / input loads (all plain copies first; the
            # X^T DMA-transposes run uninterrupted after them to avoid
            # xbar-mode serialization) ----
            wt = cpool.tile([128, 8, 6, 128], BF16)
            nc.gpsimd.dma_start(out=wt, in_=wT_d.rearrange("a b c d -> b a c d"))
            bqT = cpool.tile([128, H], F32)
            nc.gpsimd.dma_start(out=bqT, in_=bqT_d)
            bk = cpool.tile([128, 1], F32)
            nc.gpsimd.dma_start(out=bk, in_=bk_d)
            bv = cpool.tile([128, 1], F32)
            nc.gpsimd.dma_start(out=bv, in_=bv_d)
            ident = cpool.tile([128, 128], BF16)
            nc.gpsimd.dma_start(out=ident, in_=id_d)
            ones_bf = cpool.tile([128, 32], BF16)
            nc.gpsimd.dma_start(out=ones_bf, in_=ones_d)
            ones_f = cpool.tile([128, 1], F32)
            nc.gpsimd.dma_start(out=ones_f, in_=onesf_d)
            onesrow = cpool.tile([1, 128], F32)
            nc.gpsimd.dma_start(out=onesrow, in_=onesrow_d)
            wct = cpool.tile([128, CP], BF16)
            nc.gpsimd.dma_start(out=wct, in_=wcT_d)
            bcT = cpool.tile([128, NCH], F32)
            nc.gpsimd.dma_start(out=bcT, in_=bcT_d)
            # X^T per-third tiles (separate tiles -> no false deps against
            # projection reads of earlier thirds), third-major order
            xts = []
            for ti, (base, size) in enumerate(HALVES):
                xt_t = big.tile([128, 8, size], BF16, name=f"xt{ti}")
                xts.append(xt_t)
                for dc in range(8):
                    nc.sync.dma_start(
                        out=xt_t[:, dc, :],
                        in_=xT_d[base:base + size, dc * 128:(dc + 1) * 128],
                        transpose=True,
                    )

            keys_bf = big.tile([128, BSP], BF16)
            vals_bf = big.tile([128, BSP], BF16)
            prod_bf = big.tile([128, H, BSP], BF16)
            score_wide = big.tile([128, BSP], F32)

            with (
                tc.tile_pool(name="projps", bufs=2, space="PSUM") as projps,
                tc.tile_pool(name="scoreps", bufs=1, space="PSUM") as scoreps,
            ):
                def proj_part(j, ti, size, pp):
                    for dc in range(8):
                        for lo in range(0, size, 512):
                            hi = min(lo + 512, size)
                            nc.tensor.matmul(
                                pp[:, lo:hi],
                                wt[:, dc, j, :],
                                xts[ti][:, dc, lo:hi],
                                start=(dc == 0), stop=(dc == 7),
                            )

                # projections, third-major: keys, vals, then q heads
                for ti, (base, size) in enumerate(HALVES):
                    for j in range(6):
                        pp = projps.tile([128, 1216], F32, tag="proj")
                        proj_part(j, ti, size, pp)
                        if j == 0:
                            nc.scalar.activation(
                                keys_bf[:, base:base + size], pp[:, 0:size],
                                ACTF.Identity, bias=bk)
                        elif j == 1:
                            nc.scalar.activation(
                                vals_bf[:, base:base + size], pp[:, 0:size],
                                ACTF.Identity, bias=bv)
                        else:
                            h = j - 2
                            nc.vector.scalar_tensor_tensor(
                                out=prod_bf[:, h, base:base + size],
                                in0=pp[:, 0:size],
                                scalar=bqT[:, h:h + 1],
                                in1=keys_bf[:, base:base + size],
                                op0=ALU.add, op1=ALU.mult,
                            )

                # vals^T for all batches (overlaps on PE)
                vn_all = big.tile([128, BPC, 3, 128], BF16)
                for b in range(BPC):
                    vn_ps = scoreps.tile([128, 3, 128], BF16, tag="vnps",
                                         bufs=1)
                    for k, (lo, sz) in enumerate(SCH):
                        nc.tensor.matmul(
                            vn_ps[0:sz, k, :],
                            vals_bf[:, b * S + lo: b * S + lo + sz],
                            ident,
                            is_transpose=True, start=True, stop=True,
                        )
                    nc.vector.tensor_copy(
                        vn_all[:, b, 0:2, :], vn_ps[:, 0:2, :])
                    nc.vector.tensor_copy(
                        vn_all[0:44, b, 2, :], vn_ps[0:44, 2, :])

                # score: partition-reduce products via ones-matmul.
                # PE can only place outputs at partitions {0,32,64}; head 3
                # goes to its own tile and is drained into row 96.
                for ci in range(0, BSP, 512):
                    ce = min(ci + 512, BSP)
                    sp = projps.tile([128, 512], F32, tag="proj")
                    spb = scoreps.tile([32, 512], F32, tag="scoreb", bufs=1)
                    for h in range(3):
                        nc.tensor.matmul(
                            sp[32 * h:32 * h + 32, 0:ce - ci],
                            ones_bf,
                            prod_bf[:, h, ci:ce],
                            start=True, stop=True,
                        )
                    nc.tensor.matmul(
                        spb[:, 0:ce - ci], ones_bf, prod_bf[:, 3, ci:ce],
                        start=True, stop=True,
                    )
                    nc.vector.tensor_copy(
                        score_wide[0:65, ci:ce], sp[0:65, 0:ce - ci])
                    nc.vector.tensor_copy(
                        score_wide[96:97, ci:ce], spb[0:1, 0:ce - ci])

            # compact rows {0,32,64,96} x [b, s] -> [hb, s] (hb = h*BPC + b)
            score_hb = small.tile([32, S], F32, tag="shb")
            nc.sync.dma_start(
                out=score_hb,
                in_=score_wide[0::32, 0:BS].rearrange("h (b s) -> h b s", b=BPC),
            )

            # ---- softmax over s (free axis) ----
            lnwarm = small.tile([1, 1], F32, tag="lnwarm")
            nc.scalar.activation(lnwarm, onesrow[:, 0:1], ACTF.Ln)
            mx = small.tile([32, 1], F32, tag="mx")
            nc.vector.tensor_reduce(mx, score_hb, axis=AX.X, op=ALU.max)
            nbias = small.tile([32, 1], F32, tag="nb")
            nc.vector.tensor_scalar_mul(nbias, mx, -SCALE)
            expv = small.tile([32, S], F32, tag="expv")
            denom = small.tile([32, 1], F32, tag="den")
            nc.scalar.activation(
                expv, score_hb, ACTF.Exp, bias=nbias, scale=SCALE,
                accum_out=denom,
            )
            rden = small.tile([32, 1], F32, tag="rden")
            nc.vector.reciprocal(rden, denom)
            p_attn = small.tile([32, S], F32, tag="pattn")
            nc.vector.tensor_scalar_mul(p_attn, expv, rden)
            p_attn_bf = small.tile([32, S], BF16, tag="pattnb")
            nc.vector.tensor_copy(p_attn_bf, p_attn)
            score_sc = small.tile([32, S], F32, tag="scsc")
            nc.scalar.mul(score_sc, score_hb, SCALE)
            nc.sync.dma_start(out=sc_o, in_=score_sc)
            nc.sync.dma_start(out=aw_o, in_=p_attn)

            with (
                tc.tile_pool(name="tps", bufs=1, space="PSUM") as tps,
                tc.tile_pool(name="wps", bufs=2, space="PSUM") as wps,
            ):
                # ---- p_attn^T: [32, s] -> [s, 32] (3 chunks) ----
                pat_ps = tps.tile([128, 3, 32], BF16, tag="patps")
                for k, (lo, sz) in enumerate(SCH):
                    nc.tensor.matmul(
                        pat_ps[0:sz, k, :],
                        p_attn_bf[:, lo:lo + sz],
                        ident[0:32, 0:32],
                        is_transpose=True, start=True, stop=True,
                    )
                p_attn_T = small.tile([128, 3, 32], BF16, tag="patT")
                nc.vector.tensor_copy(p_attn_T[:, 0:2, :], pat_ps[:, 0:2, :])
                nc.vector.tensor_copy(p_attn_T[0:44, 2, :], pat_ps[0:44, 2, :])

                # ---- ws per batch (vals already transposed) ----
                ws_T = cpool.tile([128, 32], BF16)   # columns bh = 4*b + h
                for b in range(BPC):
                    wp = wps.tile([128, 32], F32, tag="wsps")
                    for k, (lo, sz) in enumerate(SCH):
                        nc.tensor.matmul(
                            wp[:, 0:4],
                            vn_all[0:sz, b, k, :],
                            p_attn_T[0:sz, k, b::BPC],
                            start=(k == 0), stop=(k == 2),
                        )
                    nc.scalar.activation(
                        ws_T[:, 4 * b:4 * b + 4], wp[:, 0:4], ACTF.Relu)

                # ---- logits directly in [c, bh] ----
                logitsT = big.tile([128, NCH, 32], F32)
                for ct in range(NCH):
                    lp = wps.tile([128, 32], F32, tag="lgps")
                    nc.tensor.matmul(
                        lp, wct[:, ct * 128:(ct + 1) * 128], ws_T,
                        start=True, stop=True,
                    )
                    nc.vector.tensor_scalar_add(
                        logitsT[:, ct, :], lp, bcT[:, ct:ct + 1])

                # ---- layernorm over c ----
                rowsum = work.tile([128, 32], F32, tag="rsum")
                nc.vector.tensor_reduce(
                    rowsum, logitsT.rearrange("p a b -> p b a"),
                    axis=AX.X, op=ALU.add)
                msum = tps.tile([1, 32], F32, tag="stat")
                nc.tensor.matmul(msum, ones_f, rowsum, start=True, stop=True)
                mean = small.tile([1, 32], F32, tag="mean")
                nc.vector.tensor_scalar_mul(mean, msum, 1.0 / C)
                mb_ps = tps.tile([128, 32], F32, tag="bcast")
                nc.tensor.matmul(mb_ps, onesrow, mean, start=True, stop=True)
                mean_b = work.tile([128, 32], F32, tag="meanb")
                nc.vector.tensor_copy(mean_b, mb_ps)
                nc.vector.tensor_tensor(
                    out=logitsT, in0=logitsT,
                    in1=mean_b.unsqueeze(1).broadcast_to([128, NCH, 32]),
                    op=ALU.subtract)
                sq = big.tile([128, NCH, 32], F32)
                nc.vector.tensor_tensor(out=sq, in0=logitsT, in1=logitsT,
                                        op=ALU.mult)
                sqsum = work.tile([128, 32], F32, tag="sqsum")
                nc.vector.tensor_reduce(
                    sqsum, sq.rearrange("p a b -> p b a"),
                    axis=AX.X, op=ALU.add)
                vsum = tps.tile([1, 32], F32, tag="stat")
                nc.tensor.matmul(vsum, ones_f, sqsum, start=True, stop=True)
                # pad rows (CP - C of them) each contributed (0 - mean)^2
                m2 = small.tile([1, 32], F32, tag="m2")
                nc.vector.tensor_mul(m2, mean, mean)
                nc.vector.tensor_scalar_mul(m2, m2, float(CP - C))
                ssc = small.tile([1, 32], F32, tag="ssc")
                nc.vector.tensor_sub(ssc, vsum, m2)
                var = small.tile([1, 32], F32, tag="var")
                nc.vector.tensor_scalar_mul(var, ssc, 1.0 / (C - 1))
                # std = exp(0.5*ln(var)); inv = 1/(std + eps)
                lnv = small.tile([1, 32], F32, tag="lnv")
                nc.scalar.activation(lnv, var, ACTF.Ln)
                std = small.tile([1, 32], F32, tag="std")
                nc.scalar.activation(std, lnv, ACTF.Exp, scale=0.5)
                stde = small.tile([1, 32], F32, tag="stde")
                nc.vector.tensor_scalar_add(stde, std, EPS)
                inv = small.tile([1, 32], F32, tag="inv")
                nc.vector.reciprocal(inv, stde)
                ib_ps = tps.tile([128, 32], F32, tag="bcast")
                nc.tensor.matmul(ib_ps, onesrow, inv, start=True, stop=True)
                inv_b = work.tile([128, 32], F32, tag="invb")
                nc.vector.tensor_copy(inv_b, ib_ps)
                nc.vector.tensor_tensor(
                    out=logitsT, in0=logitsT,
                    in1=inv_b.unsqueeze(1).broadcast_to([128, NCH, 32]),
                    op=ALU.mult)

            # ---- max / argmax over h; sigmoid ----
            zv = logitsT.rearrange("p a (b h) -> p a b h", h=H)
            m = work.tile([128, NCH, BPC], F32, tag="m")
            nc.vector.tensor_reduce(m, zv, axis=AX.X, op=ALU.max)
            ge = big.tile([128, NCH, BPC, H], F32)
            for h in range(3):
                nc.vector.tensor_tensor(
                    out=ge[:, :, :, h], in0=zv[:, :, :, h], in1=m, op=ALU.is_ge)
                nc.vector.tensor_scalar_mul(
                    ge[:, :, :, h], ge[:, :, :, h], float(3 - h))
            r = work.tile([128, NCH, BPC], F32, tag="r")
            nc.vector.tensor_reduce(
                r, ge[:, :, :, 0:3], axis=AX.X, op=ALU.max)
            idx = work.tile([128, NCH, BPC], I32, tag="idx")
            nc.scalar.activation(idx, r, ACTF.Copy, scale=-1.0, bias=3.0)
            nc.sync.dma_start(out=ix_o, in_=idx)
            # sigmoid(m) = 1 / (1 + exp(-m))
            em = work.tile([128, NCH, BPC], F32, tag="em")
            nc.scalar.activation(em, m, ACTF.Exp, scale=-1.0)
            nc.vector.tensor_scalar_add(em, em, 1.0)
            probs = work.tile([128, NCH, BPC], F32, tag="probs")
            nc.vector.reciprocal(probs, em)
            nc.sync.dma_start(out=pr_o, in_=probs)

    nc.compile()
    return nc


def _get_program():
    if "nc" not in _CACHE:
        _CACHE["nc"] = _build_program()
    return _CACHE["nc"]


def _prep_inputs(seg_features, Wq, bq, Wk, bk, Wv, bv, Wc, bc):
    import ml_dtypes

    BF = ml_dtypes.bfloat16
    # stacked weights, order [keys, vals, q0..q3]; W^T layout [8, 128, 6, 128]
    wstack = np.stack([Wk, Wv, Wq[0], Wq[1], Wq[2], Wq[3]], axis=0)
    wT = np.ascontiguousarray(wstack.transpose(2, 0, 1))   # [1024, 6, 128]
    wT = wT.reshape(8, 128, 6, 128).astype(BF)
    wcT = np.zeros((128, CP), np.float32)
    wcT[:, :C] = Wc.T
    wcT = wcT.astype(BF)
    bcT = np.zeros((CP,), np.float32)
    bcT[:C] = bc
    bcT = np.ascontiguousarray(bcT.reshape(NCH, 128).T)
    common = {
        "wT_in": wT,
        "wcT_in": wcT,
        "bqT_in": np.ascontiguousarray(bq.T).astype(np.float32),
        "bk_in": bk.reshape(128, 1).astype(np.float32),
        "bv_in": bv.reshape(128, 1).astype(np.float32),
        "bcT_in": bcT.astype(np.float32),
        "id_in": np.eye(128, dtype=np.float32).astype(BF),
        "ones_in": np.ones((128, 32), np.float32).astype(BF),
        "onesf_in": np.ones((128, 1), np.float32),
        "onesrow_in": np.ones((1, 128), np.float32),
    }
    in_maps = []
    for i in range(NCORES):
        xs = seg_features[i * BPC:(i + 1) * BPC].reshape(BS, D)
        xp = np.zeros((BSP, D), np.float32)
        xp[:BS] = xs
        m = dict(common)
        m["xT_in"] = xp.astype(BF)
        in_maps.append(m)
    return in_maps


def _gather(res):
    vid_probs = np.empty((B, C), np.float32)
    attn_idc = np.empty((B, C), np.int32)
    scores = np.empty((B, S, H), np.float32)
    attn_w = np.empty((B, S, H), np.float32)
    for i, r in enumerate(res):
        sl = slice(i * BPC, (i + 1) * BPC)
        vid_probs[sl] = r["probs_out"].transpose(2, 1, 0).reshape(BPC, CP)[:, :C]
        attn_idc[sl] = r["idx_out"].transpose(2, 1, 0).reshape(BPC, CP)[:, :C]
        scores[sl] = r["scores_out"].reshape(H, BPC, S).transpose(1, 2, 0)
        attn_w[sl] = r["attnw_out"].reshape(H, BPC, S).transpose(1, 2, 0)
    return vid_probs, attn_idc, scores, attn_w


def _conv_loss(Wc, bc):
    cp = (Wc.sum(axis=-1) + bc).astype(np.float64)
    cp = cp - cp.max()
    e = np.exp(cp)
    p = e / e.sum()
    stdv = math.sqrt(float(((p - p.mean()) ** 2).sum()) / (C - 1))
    return np.float32(B * min(max(stdv, 1e-9), 1e9))


def kernel(seg_features, Wq, bq, Wk, bk, Wv, bv, Wc, bc, ln_a, ln_b,
           _trace=False):
    from concourse import bass_utils

    seg_features = np.asarray(seg_features, np.float32)
    Wq = np.asarray(Wq, np.float32)
    bq = np.asarray(bq, np.float32)
    Wk = np.asarray(Wk, np.float32)
    bk = np.asarray(bk, np.float32)
    Wv = np.asarray(Wv, np.float32)
    bv = np.asarray(bv, np.float32)
    Wc = np.asarray(Wc, np.float32)
    bc = np.asarray(bc, np.float32)
    assert np.all(np.asarray(ln_a) == 1.0) and np.all(np.asarray(ln_b) == 0.0), (
        "device fast-path assumes identity layernorm affine"
    )

    nc = _get_program()
    in_maps = _prep_inputs(seg_features, Wq, bq, Wk, bk, Wv, bv, Wc, bc)
    res = bass_utils.run_bass_kernel_spmd(
        nc, in_maps, core_ids=list(range(NCORES)), trace=_trace)

    vid_probs, attn_idc, scores, attn_w = _gather(res.results)
    out = (vid_probs, attn_idc, scores, attn_w, _conv_loss(Wc, bc))
    if _trace:
        return out, res
    return out


# revision 13
# speedup vs baseline: 1.0976x; 1.0178x over previous
"""Self-contained Trainium2 Bass kernel for nn_Classifier_6682969112951.

Model (per batch b): 4-head additive attention pooling over S=300 tokens
(D=1024 -> P=128 projections), conv classifier (C=3862), layernorm over C,
max/argmax over heads, sigmoid. Outputs:
  (vid_probs [64,3862] f32, attn_idc [64,3862] i32,
   scores [64,300,4] f32, attn_weights [64,300,4] f32, conv_loss scalar f32)

Sharding: data-parallel over batch across 8 NeuronCores (8 batches/core),
weights replicated. conv_loss depends only on (Wc, bc) -> host numpy.

Device strategy (bf16 matmuls, fp32 accumulation / softmax / layernorm):
  - X^T arrives via hardware DMA-transpose from DRAM (host pre-casts to
    bf16 and pads (b,s)=2400 rows -> 2432).
  - Projections: stationary W^T chunks [128d,128p], moving X^T, accumulated
    over 8 d-chunks in PSUM, drained with fused bias add.
  - score = sum_p (q+bq)*keys: elementwise product (DVE) then ones-matmul
    partition reduction; the 4 head rows land at PSUM partitions {0,32,64,96}
    and are compacted to one [32(hb),300] tile by an SBUF->SBUF DMA.
  - softmax on [32,300]; ws via per-batch matmuls against transposed vals;
    logits directly in [c,bh] layout (Wc^T chunks stationary); LN stats via
    DVE pre-reduce + ones-matmul partition sums; normalize with stride-0
    broadcast; max/argmax over heads via is_ge arithmetic; sigmoid via
    exp + reciprocal (keeps ACT on one table set).
  - probs/idx leave the device in [c_lo, ch, b] layout; host permutes.
"""
import math

import numpy as np

B, S, D, P, H, C = 64, 300, 1024, 128, 4, 3862
NCORES = 8
BPC = B // NCORES          # batches per core
BS = BPC * S               # 2400
BSP = 2432                 # padded (b,s) extent: 19*128, multiple of 16
CP = 3968                  # padded C: 31*128
NCH = CP // 128            # 31 c-chunks
SCALE = 1.0 / math.sqrt(P)
EPS = 1e-6

_CACHE = {}


def _build_program():
    import concourse.mybir as mybir
    import concourse.tile as tile
    from concourse import bacc

    F32 = mybir.dt.float32
    BF16 = mybir.dt.bfloat16
    I32 = mybir.dt.int32
    AX = mybir.AxisListType
    ALU = mybir.AluOpType
    ACTF = mybir.ActivationFunctionType

    nc = bacc.Bacc("TRN2", target_bir_lowering=False, debug=False)

    xT_d = nc.dram_tensor("xT_in", [BSP, D], BF16, kind="ExternalInput").ap()
    wT_d = nc.dram_tensor("wT_in", [8, 128, 6, 128], BF16, kind="ExternalInput").ap()
    wcT_d = nc.dram_tensor("wcT_in", [128, CP], BF16, kind="ExternalInput").ap()
    bqT_d = nc.dram_tensor("bqT_in", [128, H], F32, kind="ExternalInput").ap()
    bk_d = nc.dram_tensor("bk_in", [128, 1], F32, kind="ExternalInput").ap()
    bv_d = nc.dram_tensor("bv_in", [128, 1], F32, kind="ExternalInput").ap()
    bcT_d = nc.dram_tensor("bcT_in", [128, NCH], F32, kind="ExternalInput").ap()
    id_d = nc.dram_tensor("id_in", [128, 128], BF16, kind="ExternalInput").ap()
    ones_d = nc.dram_tensor("ones_in", [128, 32], BF16, kind="ExternalInput").ap()
    onesf_d = nc.dram_tensor("onesf_in", [128, 1], F32, kind="ExternalInput").ap()
    onesrow_d = nc.dram_tensor("onesrow_in", [1, 128], F32, kind="ExternalInput").ap()

    sc_o = nc.dram_tensor("scores_out", [32, S], F32, kind="ExternalOutput").ap()
    aw_o = nc.dram_tensor("attnw_out", [32, S], F32, kind="ExternalOutput").ap()
    pr_o = nc.dram_tensor("probs_out", [128, NCH, BPC], F32, kind="ExternalOutput").ap()
    ix_o = nc.dram_tensor("idx_out", [128, NCH, BPC], I32, kind="ExternalOutput").ap()

    HALVES = ((0, 1216), (1216, 1216))
    SCH = ((0, 128), (128, 128), (256, 44))  # per-batch s chunks

    with tile.TileContext(nc) as tc:
        with (
            tc.tile_pool(name="const", bufs=1) as cpool,
            tc.tile_pool(name="big", bufs=1) as big,
            tc.tile_pool(name="work", bufs=2) as work,
            tc.tile_pool(name="small", bufs=2) as small,
        ):
            # ---- constant / input loads (all plain copies first; the
            # X^T DMA-transposes run uninterrupted after them to avoid
            # xbar-mode serialization) ----
            wt = cpool.tile([128, 8, 6, 128], BF16)
            nc.gpsimd.dma_start(out=wt, in_=wT_d.rearrange("a b c d -> b a c d"))
            bqT = cpool.tile([128, H], F32)
            nc.gpsimd.dma_start(out=bqT, in_=bqT_d)
            bk = cpool.tile([128, 1], F32)
            nc.gpsimd.dma_start(out=bk, in_=bk_d)
            bv = cpool.tile([128, 1], F32)
            nc.gpsimd.dma_start(out=bv, in_=bv_d)
            ident = cpool.tile([128, 128], BF16)
            nc.gpsimd.dma_start(out=ident, in_=id_d)
            ones_bf = cpool.tile([128, 32], BF16)
            nc.gpsimd.dma_start(out=ones_bf, in_=ones_d)
            ones_f = cpool.tile([128, 1], F32)
            nc.gpsimd.dma_start(out=ones_f, in_=onesf_d)
            onesrow = cpool.tile([1, 128], F32)
            nc.gpsimd.dma_start(out=onesrow, in_=onesrow_d)
            wct = cpool.tile([128, CP], BF16)
            nc.gpsimd.dma_start(out=wct, in_=wcT_d)
            bcT = cpool.tile([128, NCH], F32)
            nc.gpsimd.dma_start(out=bcT, in_=bcT_d)
            # X^T per-third tiles (separate tiles -> no false deps against
            # projection reads of earlier thirds), third-major order
            xts = []
            for ti, (base, size) in enumerate(HALVES):
                xt_t = big.tile([128, 8, size], BF16, name=f"xt{ti}")
                xts.append(xt_t)
                for dc in range(8):
                    nc.sync.dma_start(
                        out=xt_t[:, dc, :],
                        in_=xT_d[base:base + size, dc * 128:(dc + 1) * 128],
                        transpose=True,
                    )

            keys_bf = big.tile([128, BSP], BF16)
            vals_bf = big.tile([128, BSP], BF16)
            prod_bf = big.tile([128, H, BSP], BF16)
            score_wide = big.tile([128, BSP], F32)

            with (
                tc.tile_pool(name="projps", bufs=2, space="PSUM") as projps,
                tc.tile_pool(name="scoreps", bufs=1, space="PSUM") as scoreps,
            ):
                def proj_part(j, ti, size, pp):
                    for dc in range(8):
                        for lo in range(0, size, 512):
                            hi = min(lo + 512, size)
                            nc.tensor.matmul(
                                pp[:, lo:hi],
                                wt[:, dc, j, :],
                                xts[ti][:, dc, lo:hi],
                                start=(dc == 0), stop=(dc == 7),
                            )

                # projections, third-major: keys, vals, then q heads
                for ti, (base, size) in enumerate(HALVES):
                    for j in range(6):
                        pp = projps.tile([128, 1216], F32, tag="proj")
                        proj_part(j, ti, size, pp)
                        if j == 0:
                            nc.scalar.activation(
                                keys_bf[:, base:base + size], pp[:, 0:size],
                                ACTF.Identity, bias=bk)
                        elif j == 1:
                            nc.scalar.activation(
                                vals_bf[:, base:base + size], pp[:, 0:size],
                                ACTF.Identity, bias=bv)
                        else:
                            h = j - 2
                            qb = work.tile([128, 1216], BF16, tag="qb")
                            nc.scalar.activation(
                                qb[:, 0:size], pp[:, 0:size],
                                ACTF.Identity, bias=bqT[:, h:h + 1])
                            nc.gpsimd.tensor_mul(
                                prod_bf[:, h, base:base + size],
                                qb[:, 0:size],
                                keys_bf[:, base:base + size])

                # vals^T for all batches (overlaps on PE)
                vn_all = big.tile([128, BPC, 3, 128], BF16)
                for b in range(BPC):
                    vn_ps = scoreps.tile([128, 3, 128], BF16, tag="vnps",
                                         bufs=1)
                    for k, (lo, sz) in enumerate(SCH):
                        nc.tensor.matmul(
                            vn_ps[0:sz, k, :],
                            vals_bf[:, b * S + lo: b * S + lo + sz],
                            ident,
                            is_transpose=True, start=True, stop=True,
                        )
                    nc.vector.tensor_copy(
                        vn_all[:, b, 0:2, :], vn_ps[:, 0:2, :])
                    nc.vector.tensor_copy(
                        vn_all[0:44, b, 2, :], vn_ps[0:44, 2, :])

                # score: partition-reduce products via ones-matmul.
                # PE can only place outputs at partitions {0,32,64}; head 3
                # goes to its own tile and is drained into row 96.
                for ci in range(0, BSP, 512):
                    ce = min(ci + 512, BSP)
                    sp = projps.tile([128, 512], F32, tag="proj")
                    spb = scoreps.tile([32, 512], F32, tag="scoreb", bufs=1)
                    for h in range(3):
                        nc.tensor.matmul(
                            sp[32 * h:32 * h + 32, 0:ce - ci],
                            ones_bf,
                            prod_bf[:, h, ci:ce],
                            start=True, stop=True,
                        )
                    nc.tensor.matmul(
                        spb[:, 0:ce - ci], ones_bf, prod_bf[:, 3, ci:ce],
                        start=True, stop=True,
                    )
                    nc.scalar.copy(score_wide[0:65, ci:ce], sp[0:65, 0:ce - ci])
                    nc.scalar.copy(score_wide[96:97, ci:ce], spb[0:1, 0:ce - ci])

            # compact rows {0,32,64,96} x [b, s] -> [hb, s] (hb = h*BPC + b)
            score_hb = small.tile([32, S], F32, tag="shb")
            nc.sync.dma_start(
                out=score_hb,
                in_=score_wide[0::32, 0:BS].rearrange("h (b s) -> h b s", b=BPC),
            )

            # ---- softmax over s (free axis) ----
            lnwarm = small.tile([1, 1], F32, tag="lnwarm")
            nc.scalar.activation(lnwarm, onesrow[:, 0:1], ACTF.Ln)
            mx = small.tile([32, 1], F32, tag="mx")
            nc.vector.tensor_reduce(mx, score_hb, axis=AX.X, op=ALU.max)
            nbias = small.tile([32, 1], F32, tag="nb")
            nc.vector.tensor_scalar_mul(nbias, mx, -SCALE)
            expv = small.tile([32, S], F32, tag="expv")
            denom = small.tile([32, 1], F32, tag="den")
            nc.scalar.activation(
                expv, score_hb, ACTF.Exp, bias=nbias, scale=SCALE,
                accum_out=denom,
            )
            rden = small.tile([32, 1], F32, tag="rden")
            nc.vector.reciprocal(rden, denom)
            p_attn = small.tile([32, S], F32, tag="pattn")
            nc.vector.tensor_scalar_mul(p_attn, expv, rden)
            p_attn_bf = small.tile([32, S], BF16, tag="pattnb")
            nc.vector.tensor_copy(p_attn_bf, p_attn)
            score_sc = small.tile([32, S], F32, tag="scsc")
            nc.scalar.mul(score_sc, score_hb, SCALE)
            nc.sync.dma_start(out=sc_o, in_=score_sc)
            nc.sync.dma_start(out=aw_o, in_=p_attn)

            with (
                tc.tile_pool(name="tps", bufs=1, space="PSUM") as tps,
                tc.tile_pool(name="wps", bufs=2, space="PSUM") as wps,
            ):
                # ---- p_attn^T: [32, s] -> [s, 32] (3 chunks) ----
                pat_ps = tps.tile([128, 3, 32], BF16, tag="patps")
                for k, (lo, sz) in enumerate(SCH):
                    nc.tensor.matmul(
                        pat_ps[0:sz, k, :],
                        p_attn_bf[:, lo:lo + sz],
                        ident[0:32, 0:32],
                        is_transpose=True, start=True, stop=True,
                    )
                p_attn_T = small.tile([128, 3, 32], BF16, tag="patT")
                nc.vector.tensor_copy(p_attn_T[:, 0:2, :], pat_ps[:, 0:2, :])
                nc.vector.tensor_copy(p_attn_T[0:44, 2, :], pat_ps[0:44, 2, :])

                # ---- ws per batch (vals already transposed) ----
                ws_T = cpool.tile([128, 32], BF16)   # columns bh = 4*b + h
                for b in range(BPC):
                    wp = wps.tile([128, 32], F32, tag="wsps")
                    for k, (lo, sz) in enumerate(SCH):
                        nc.tensor.matmul(
                            wp[:, 0:4],
                            vn_all[0:sz, b, k, :],
                            p_attn_T[0:sz, k, b::BPC],
                            start=(k == 0), stop=(k == 2),
                        )
                    nc.scalar.activation(
                        ws_T[:, 4 * b:4 * b + 4], wp[:, 0:4], ACTF.Relu)

                # ---- logits directly in [c, bh] ----
                logitsT = big.tile([128, NCH, 32], F32)
                for ct in range(NCH):
                    lp = wps.tile([128, 32], F32, tag="lgps")
                    nc.tensor.matmul(
                        lp, wct[:, ct * 128:(ct + 1) * 128], ws_T,
                        start=True, stop=True,
                    )
                    nc.vector.tensor_scalar_add(
                        logitsT[:, ct, :], lp, bcT[:, ct:ct + 1])

                # ---- layernorm over c ----
                rowsum = work.tile([128, 32], F32, tag="rsum")
                nc.vector.tensor_reduce(
                    rowsum, logitsT.rearrange("p a b -> p b a"),
                    axis=AX.X, op=ALU.add)
                msum = tps.tile([1, 32], F32, tag="stat")
                nc.tensor.matmul(msum, ones_f, rowsum, start=True, stop=True)
                mean = small.tile([1, 32], F32, tag="mean")
                nc.vector.tensor_scalar_mul(mean, msum, 1.0 / C)
                mb_ps = tps.tile([128, 32], F32, tag="bcast")
                nc.tensor.matmul(mb_ps, onesrow, mean, start=True, stop=True)
                mean_b = work.tile([128, 32], F32, tag="meanb")
                nc.vector.tensor_copy(mean_b, mb_ps)
                nc.vector.tensor_tensor(
                    out=logitsT, in0=logitsT,
                    in1=mean_b.unsqueeze(1).broadcast_to([128, NCH, 32]),
                    op=ALU.subtract)
                sq = big.tile([128, NCH, 32], F32)
                nc.vector.tensor_tensor(out=sq, in0=logitsT, in1=logitsT,
                                        op=ALU.mult)
                sqsum = work.tile([128, 32], F32, tag="sqsum")
                nc.vector.tensor_reduce(
                    sqsum, sq.rearrange("p a b -> p b a"),
                    axis=AX.X, op=ALU.add)
                vsum = tps.tile([1, 32], F32, tag="stat")
                nc.tensor.matmul(vsum, ones_f, sqsum, start=True, stop=True)
                # pad rows (CP - C of them) each contributed (0 - mean)^2
                m2 = small.tile([1, 32], F32, tag="m2")
                nc.vector.tensor_mul(m2, mean, mean)
                nc.vector.tensor_scalar_mul(m2, m2, float(CP - C))
                ssc = small.tile([1, 32], F32, tag="ssc")
                nc.vector.tensor_sub(ssc, vsum, m2)
                var = small.tile([1, 32], F32, tag="var")
                nc.vector.tensor_scalar_mul(var, ssc, 1.0 / (C - 1))
                # std = exp(0.5*ln(var)); inv = 1/(std + eps)
                lnv = small.tile([1, 32], F32, tag="lnv")
                nc.scalar.activation(lnv, var, ACTF.Ln)
                std = small.tile([1, 32], F32, tag="std")
                nc.scalar.activation(std, lnv, ACTF.Exp, scale=0.5)
                stde = small.tile([1, 32], F32, tag="stde")
                nc.vector.tensor_scalar_add(stde, std, EPS)
                inv = small.tile([1, 32], F32, tag="inv")
                nc.vector.reciprocal(inv, stde)
                ib_ps = tps.tile([128, 32], F32, tag="bcast")
                nc.tensor.matmul(ib_ps, onesrow, inv, start=True, stop=True)
                inv_b = work.tile([128, 32], F32, tag="invb")
                nc.vector.tensor_copy(inv_b, ib_ps)
                nc.vector.tensor_tensor(
                    out=logitsT, in0=logitsT,
                    in1=inv_b.unsqueeze(1).broadcast_to([128, NCH, 32]),
                    op=ALU.mult)

            # ---- max / argmax over h; sigmoid ----
            zv = logitsT.rearrange("p a (b h) -> p a b h", h=H)
            m = work.tile([128, NCH, BPC], F32, tag="m")
            nc.vector.tensor_reduce(m, zv, axis=AX.X, op=ALU.max)
            ge = big.tile([128, NCH, BPC, H], F32)
            for h in range(3):
                nc.vector.tensor_tensor(
                    out=ge[:, :, :, h], in0=zv[:, :, :, h], in1=m, op=ALU.is_ge)
                nc.vector.tensor_scalar_mul(
                    ge[:, :, :, h], ge[:, :, :, h], float(3 - h))
            r = work.tile([128, NCH, BPC], F32, tag="r")
            nc.vector.tensor_reduce(
                r, ge[:, :, :, 0:3], axis=AX.X, op=ALU.max)
            idx = work.tile([128, NCH, BPC], I32, tag="idx")
            nc.scalar.activation(idx, r, ACTF.Copy, scale=-1.0, bias=3.0)
            nc.sync.dma_start(out=ix_o, in_=idx)
            # sigmoid(m) = 1 / (1 + exp(-m))
            em = work.tile([128, NCH, BPC], F32, tag="em")
            nc.scalar.activation(em, m, ACTF.Exp, scale=-1.0)
            nc.vector.tensor_scalar_add(em, em, 1.0)
            probs = work.tile([128, NCH, BPC], F32, tag="probs")
            nc.vector.reciprocal(probs, em)
            nc.sync.dma_start(out=pr_o, in_=probs)

    nc.compile()
    return nc


def _get_program():
    if "nc" not in _CACHE:
        _CACHE["nc"] = _build_program()
    return _CACHE["nc"]


def _prep_inputs(seg_features, Wq, bq, Wk, bk, Wv, bv, Wc, bc):
    import ml_dtypes

    BF = ml_dtypes.bfloat16
    # stacked weights, order [keys, vals, q0..q3]; W^T layout [8, 128, 6, 128]
    wstack = np.stack([Wk, Wv, Wq[0], Wq[1], Wq[2], Wq[3]], axis=0)
    wT = np.ascontiguousarray(wstack.transpose(2, 0, 1))   # [1024, 6, 128]
    wT = wT.reshape(8, 128, 6, 128).astype(BF)
    wcT = np.zeros((128, CP), np.float32)
    wcT[:, :C] = Wc.T
    wcT = wcT.astype(BF)
    bcT = np.zeros((CP,), np.float32)
    bcT[:C] = bc
    bcT = np.ascontiguousarray(bcT.reshape(NCH, 128).T)
    common = {
        "wT_in": wT,
        "wcT_in": wcT,
        "bqT_in": np.ascontiguousarray(bq.T).astype(np.float32),
        "bk_in": bk.reshape(128, 1).astype(np.float32),
        "bv_in": bv.reshape(128, 1).astype(np.float32),
        "bcT_in": bcT.astype(np.float32),
        "id_in": np.eye(128, dtype=np.float32).astype(BF),
        "ones_in": np.ones((128, 32), np.float32).astype(BF),
        "onesf_in": np.ones((128, 1), np.float32),
        "onesrow_in": np.ones((1, 128), np.float32),
    }
    in_maps = []
    for i in range(NCORES):
        xs = seg_features[i * BPC:(i + 1) * BPC].reshape(BS, D)
        xp = np.zeros((BSP, D), np.float32)
        xp[:BS] = xs
        m = dict(common)
        m["xT_in"] = xp.astype(BF)
        in_maps.append(m)
    return in_maps


def _gather(res):
    vid_probs = np.empty((B, C), np.float32)
    attn_idc = np.empty((B, C), np.int32)
    scores = np.empty((B, S, H), np.float32)
    attn_w = np.empty((B, S, H), np.float32)
    for i, r in enumerate(res):
        sl = slice(i * BPC, (i + 1) * BPC)
        vid_probs[sl] = r["probs_out"].transpose(2, 1, 0).reshape(BPC, CP)[:, :C]
        attn_idc[sl] = r["idx_out"].transpose(2, 1, 0).reshape(BPC, CP)[:, :C]
        scores[sl] = r["scores_out"].reshape(H, BPC, S).transpose(1, 2, 0)
        attn_w[sl] = r["attnw_out"].reshape(H, BPC, S).transpose(1, 2, 0)
    return vid_probs, attn_idc, scores, attn_w


def _conv_loss(Wc, bc):
    cp = (Wc.sum(axis=-1) + bc).astype(np.float64)
    cp = cp - cp.max()
    e = np.exp(cp)
    p = e / e.sum()
    stdv = math.sqrt(float(((p - p.mean()) ** 2).sum()) / (C - 1))
    return np.float32(B * min(max(stdv, 1e-9), 1e9))


def kernel(seg_features, Wq, bq, Wk, bk, Wv, bv, Wc, bc, ln_a, ln_b,
           _trace=False):
    from concourse import bass_utils

    seg_features = np.asarray(seg_features, np.float32)
    Wq = np.asarray(Wq, np.float32)
    bq = np.asarray(bq, np.float32)
    Wk = np.asarray(Wk, np.float32)
    bk = np.asarray(bk, np.float32)
    Wv = np.asarray(Wv, np.float32)
    bv = np.asarray(bv, np.float32)
    Wc = np.asarray(Wc, np.float32)
    bc = np.asarray(bc, np.float32)
    assert np.all(np.asarray(ln_a) == 1.0) and np.all(np.asarray(ln_b) == 0.0), (
        "device fast-path assumes identity layernorm affine"
    )

    nc = _get_program()
    in_maps = _prep_inputs(seg_features, Wq, bq, Wk, bk, Wv, bv, Wc, bc)
    res = bass_utils.run_bass_kernel_spmd(
        nc, in_maps, core_ids=list(range(NCORES)), trace=_trace)

    vid_probs, attn_idc, scores, attn_w = _gather(res.results)
    out = (vid_probs, attn_idc, scores, attn_w, _conv_loss(Wc, bc))
    if _trace:
        return out, res
    return out
